# revision 14
# baseline (speedup 1.0000x reference)
"""MetaJanusAttention Trainium2 Bass kernel.

Problem (per batch b):
    J        = x @ Wj.T
    scores   = ||J_t||^2 / sqrt(E)          (= x^T (Wj^T Wj) x / sqrt(E) rowwise)
    attn     = causal_softmax(scores_t * scores_s)    # head-shared [T, T]
    val      = x @ Wv.T                                # [T, E]
    out      = (attn @ val) @ Wo.T                     # [T, E]

Sharding: 8 cores = 4 batches x 2 head-halves. Each core handles one
batch and one 512-wide half of the val/Wo head dimension; the host sums
the two partial outputs per batch.

Per-core layouts are chosen so no on-device transposes are needed:
inputs arrive pre-transposed ([feature, token]) and every matmul's
contraction dim lands on partitions. Scores use the "G-path": the host
precomputes G = Wj^T @ Wj in fp64; the device computes
xG^T = G_off @ x^T in f32r plus the diagonal term d_e * x^2 exactly on
the Scalar/Vector engines (the diagonal dominates |scores| and f32r
rounding of it would be amplified by the exp).

SBUF is tight: the runtime reserves top-of-SBUF (load fails above
~167 KB/partition), so ctx reuses the dead xT buffer, the wv slot is
recycled for masks and then Wo, and all row vectors / partition
broadcasts are chunk-sized and bounced through DRAM.
"""

import numpy as np

import concourse.bass as bass
import concourse.mybir as mybir
import concourse.tile as tile
from concourse import bacc
from concourse.bass_utils import run_bass_kernel_spmd

dt = mybir.dt
F32 = dt.float32
F32R = dt.float32r
ALU = mybir.AluOpType
ACTF = mybir.ActivationFunctionType

B, T, E, H, D = 4, 2048, 1024, 16, 64
P = 128
HALF = E // 2  # per-core head-dim slice (8 heads * 64)
KO = E // P  # 8 k-tiles over the E contraction dim
JO = HALF // P  # 4 tiles over the per-core head dim
CH = 512  # free-dim chunk (fp32 moving-operand max)
NCH = T // CH  # 4
STT = T // P  # 16 s-tiles
NEG = -1.0e9
RSQRT_E = 1.0 / np.sqrt(np.float32(E))

_cache: dict = {}


def _build_nc():
    nc = bacc.Bacc(None)

    xT = nc.declare_dram_parameter("xT", [E, T], F32R, isOutput=False)
    goff = nc.declare_dram_parameter("goff", [E, E], F32R, isOutput=False)
    dvec = nc.declare_dram_parameter("dvec", [P, KO], F32, isOutput=False)
    wvT = nc.declare_dram_parameter("wvT", [E, HALF], F32R, isOutput=False)
    woT = nc.declare_dram_parameter("woT", [HALF, E], F32R, isOutput=False)
    masks = nc.declare_dram_parameter("masks", [P, NCH, CH], F32, isOutput=False)
    onesc = nc.declare_dram_parameter("onesc", [P, P], F32R, isOutput=False)
    outT = nc.declare_dram_parameter("outT", [E, T], F32, isOutput=True)

    xT_t = xT.rearrange("(k p) t -> p k t", p=P)
    goff_t = goff.rearrange("(k p) e -> p k e", p=P)
    wvT_t = wvT.rearrange("(k p) h -> p k h", p=P)
    woT_t = woT.rearrange("(j p) e -> p j e", p=P)
    outT_t = outT.rearrange("(i p) t -> p i t", p=P)

    sc_dram = nc.dram_tensor("sc_bounce", [T], F32)
    c_dram = nc.dram_tensor("c_bounce", [T], F32)
    r_dram = nc.dram_tensor("r_bounce", [T], F32)

    with tile.TileContext(nc) as tc:
        with (
            tc.tile_pool(name="resA", bufs=1) as resA,  # xT; reused as ctx in C
            tc.tile_pool(name="gstr", bufs=2) as gstr,  # streamed goff blocks
            tc.tile_pool(name="resC", bufs=1) as resC,  # wvT -> masks -> woT; val
            tc.tile_pool(name="repch", bufs=2) as repch,  # chunked broadcasts
            tc.tile_pool(name="repc1", bufs=1) as repc1,  # crep (single buf)
            tc.tile_pool(name="rowc", bufs=1) as rowc,  # chunked [1, CH] rows
            tc.tile_pool(name="rowp", bufs=2) as rowp,  # prefix-max chain rows
            tc.tile_pool(name="small", bufs=1) as small,
            tc.tile_pool(name="work", bufs=2) as work,
            tc.tile_pool(name="pt", bufs=3) as ptp,
            tc.tile_pool(name="ps", bufs=2, space="PSUM") as ps,
            tc.tile_pool(name="psacc", bufs=1, space="PSUM") as psacc,
        ):
            # ---------- resident loads ----------
            xt = resA.tile([P, KO, T], F32R)
            nc.sync.dma_start(xt[:], xT_t)
            wvt = resC.tile([P, KO, HALF], F32R, tag="wv")
            nc.sync.dma_start(wvt[:], wvT_t)
            dcol = small.tile([P, KO], F32)
            nc.sync.dma_start(dcol[:], dvec[:])
            ones2d = small.tile([P, P], F32R)
            nc.sync.dma_start(ones2d[:], onesc[:])

            # ---------- phase A: scores ----------
            # qs[c] accumulates scores_unnorm[., chunk] over all e-tiles
            qs = [
                psacc.tile([P, CH], F32, tag=f"acc{c}", name=f"qs{c}")
                for c in range(NCH)
            ]
            for i in range(KO):
                gblk = gstr.tile([P, KO, P], F32R, tag="gblk")
                nc.sync.dma_start(gblk[:], goff_t[:, :, i * P : (i + 1) * P])
                for c in range(NCH):
                    pxg = ps.tile([P, CH], F32, tag="mm")
                    for k in range(KO):
                        nc.tensor.matmul(
                            pxg[:],
                            gblk[:, k, :],
                            xt[:, k, c * CH : (c + 1) * CH],
                            start=(k == 0),
                            stop=(k == KO - 1),
                        )
                    # exact (unrounded) x slice for the scalar path
                    xf = work.tile([P, CH], F32, tag="xf")
                    nc.sync.dma_start(
                        xf[:], xT_t[:, i, c * CH : (c + 1) * CH].bitcast(F32)
                    )
                    # xg = d_e * x + xG_off   (exact diagonal added in fp32)
                    xg = work.tile([P, CH], F32, tag="xg")
                    nc.vector.scalar_tensor_tensor(
                        xg[:], xf[:], dcol[:, i : i + 1], pxg[:], ALU.mult, ALU.add
                    )
                    # prod = x * xg  -> f32r for the ones-matmul reduction
                    prod = ptp.tile([P, CH], F32R, tag="pt")
                    nc.vector.tensor_tensor(prod[:], xf[:], xg[:], ALU.mult)
                    nc.tensor.matmul(
                        qs[c][:],
                        ones2d[:],
                        prod[:],
                        start=(i == 0),
                        stop=(i == KO - 1),
                    )

            # scores / prefix-max / shift, chunk-chained; rows bounce via DRAM
            pm_prev = None
            for c in range(NCH):
                srow = rowc.tile([1, CH], F32, tag="srow", name=f"srow{c}")
                nc.scalar.activation(
                    srow[:], qs[c][0:1, :], ACTF.Copy, scale=RSQRT_E
                )
                prow = rowp.tile([1, CH], F32, tag="prow", name=f"prow{c}")
                init = -3.0e38 if pm_prev is None else pm_prev[:, CH - 1 : CH]
                nc.vector.tensor_tensor_scan(
                    prow[:], srow[:], srow[:], init, ALU.max, ALU.max
                )
                crw = rowc.tile([1, CH], F32, tag="crow", name=f"crow{c}")
                nc.vector.tensor_tensor(crw[:], srow[:], prow[:], ALU.mult)
                nc.sync.dma_start(sc_dram[None, c * CH : (c + 1) * CH], srow[:])
                nc.sync.dma_start(c_dram[None, c * CH : (c + 1) * CH], crw[:])
                pm_prev = prow

            # scores in partition-major tile layout: scorePT[p, o] = s[128 o + p]
            scorePT = small.tile([P, STT], F32)
            nc.sync.dma_start(scorePT[:], sc_dram.rearrange("(o p) -> p o", p=P))

            # ---------- phase B: val = x @ Wv_half.T  -> [s, hd] ----------
            val = resC.tile([P, STT, HALF], F32R, tag="val")
            for st in range(STT):
                pv = ps.tile([P, HALF], F32, tag="mm")
                for k in range(KO):
                    nc.tensor.matmul(
                        pv[:],
                        xt[:, k, st * P : (st + 1) * P],
                        wvt[:, k, :],
                        start=(k == 0),
                        stop=(k == KO - 1),
                    )
                nc.any.tensor_copy(out=val[:, st, :], in_=pv[:])

            # ---------- phase C: attention ----------
            # ctx reuses the (now dead) xT buffer: [P, JO, T] slice of xt
            ctx = xt[:, 0:JO, :]
            # masks reuse the (now dead) wvT slot
            maskt = resC.tile([P, NCH, CH], F32, tag="wv")
            nc.sync.dma_start(maskt[:], masks[:])
            for c in range(NCH):
                srep = repch.tile([P, CH], F32, tag="sch", name=f"srep{c}")
                crep = repc1.tile([P, CH], F32, tag="cch", name=f"crep{c}")
                nc.sync.dma_start(
                    srep[:],
                    sc_dram[None, c * CH : (c + 1) * CH].to_broadcast((P, CH)),
                )
                nc.sync.dma_start(
                    crep[:],
                    c_dram[None, c * CH : (c + 1) * CH].to_broadcast((P, CH)),
                )
                pctx = [
                    psacc.tile([P, CH], F32, tag=f"acc{j}", name=f"pctx{c}_{j}")
                    for j in range(JO)
                ]
                pden = psacc.tile([P, CH], F32, tag="den", name=f"pden{c}")
                n_st = 4 * c + 4
                for st in range(n_st):
                    # arg = s_s * s_t - c_t  (+ causal mask on diagonal tiles)
                    arg = work.tile([P, CH], F32, tag="xf")
                    nc.vector.scalar_tensor_tensor(
                        arg[:],
                        srep[:],
                        scorePT[:, st : st + 1],
                        crep[:],
                        ALU.mult,
                        ALU.subtract,
                    )
                    diag = st - 4 * c
                    if diag >= 0:
                        nc.vector.tensor_tensor(
                            arg[:], arg[:], maskt[:, diag, :], ALU.add
                        )
                    pt_ = ptp.tile([P, CH], F32R, tag="pt")
                    nc.scalar.activation(pt_[:], arg[:], ACTF.Exp)
                    for j in range(JO):
                        nc.tensor.matmul(
                            pctx[j][:],
                            val[:, st, j * P : (j + 1) * P],
                            pt_[:],
                            start=(st == 0),
                            stop=(st == n_st - 1),
                        )
                    nc.tensor.matmul(
                        pden[:],
                        ones2d[:],
                        pt_[:],
                        start=(st == 0),
                        stop=(st == n_st - 1),
                    )
                for j in range(JO):
                    nc.any.tensor_copy(
                        out=ctx[:, j, c * CH : (c + 1) * CH], in_=pctx[j][:]
                    )
                # phase D chunk-wise: recip of den with one Newton step
                drow = rowc.tile([1, CH], F32, tag="srow", name=f"drow{c}")
                nc.any.tensor_copy(out=drow[:], in_=pden[0:1, :])
                rrow = rowp.tile([1, CH], F32, tag="prow", name=f"rrow{c}")
                nc.vector.reciprocal(rrow[:], drow[:])
                trow = rowc.tile([1, CH], F32, tag="crow", name=f"trow{c}")
                nc.vector.tensor_tensor(trow[:], drow[:], rrow[:], ALU.mult)
                nc.vector.tensor_scalar(
                    out=trow[:], in0=trow[:], scalar1=-1.0, scalar2=2.0,
                    op0=ALU.mult, op1=ALU.add,
                )
                nc.vector.tensor_tensor(rrow[:], rrow[:], trow[:], ALU.mult)
                nc.sync.dma_start(r_dram[None, c * CH : (c + 1) * CH], rrow[:])

            # ---------- phase E: out = (ctx/den) @ Wo_half.T  -> [e, t] ----------
            # woT reuses the wv slot (masks are dead after phase C)
            wot = resC.tile([P, JO, E], F32R, tag="wv")
            nc.sync.dma_start(wot[:], woT_t)
            for c in range(NCH):
                rrep = repch.tile([P, CH], F32, tag="sch", name=f"rrep{c}")
                nc.sync.dma_start(
                    rrep[:],
                    r_dram[None, c * CH : (c + 1) * CH].to_broadcast((P, CH)),
                )
                for i in range(KO):
                    po = ps.tile([P, CH], F32, tag="mm")
                    for j in range(JO):
                        nc.tensor.matmul(
                            po[:],
                            wot[:, j, i * P : (i + 1) * P],
                            ctx[:, j, c * CH : (c + 1) * CH],
                            start=(j == 0),
                            stop=(j == JO - 1),
                        )
                    ot = work.tile([P, CH], F32, tag="xg")
                    nc.vector.tensor_tensor(ot[:], po[:], rrep[:], ALU.mult)
                    nc.sync.dma_start(outT_t[:, i, c * CH : (c + 1) * CH], ot[:])

    nc.compile()
    return nc


def _host_prep(x, Wj, Wv, Wo):
    """Per-core input maps. Core cid: batch cid//2, head-half cid%2."""
    G = Wj.T.astype(np.float64) @ Wj.astype(np.float64)
    d = np.diag(G).copy()
    goff = (G - np.diag(d)).astype(np.float32)
    d32 = d.astype(np.float32)
    # dvec in partition-major layout: dvec[p, k] = d[128 k + p]
    dvec = np.ascontiguousarray(d32.reshape(KO, P).T)

    # additive causal masks for the 4 diagonal s-tile positions
    masks = np.zeros((P, NCH, CH), dtype=np.float32)
    for pos in range(NCH):
        r = np.arange(P)[:, None] + P * pos
        cidx = np.arange(CH)[None, :]
        masks[:, pos, :] = np.where(r <= cidx, 0.0, NEG)

    xTs = [np.ascontiguousarray(x[b].T) for b in range(B)]
    in_maps = []
    for cid in range(2 * B):
        b, hh = divmod(cid, 2)
        rowsl = slice(hh * HALF, (hh + 1) * HALF)
        in_maps.append(
            {
                "xT": xTs[b],
                "goff": goff,
                "dvec": dvec,
                "wvT": np.ascontiguousarray(Wv[rowsl, :].T),
                "woT": np.ascontiguousarray(Wo[:, rowsl].T),
                "masks": masks,
                "onesc": np.ones((P, P), dtype=np.float32),
            }
        )
    return in_maps


def kernel(x, Wj, Wv, Wo):
    x = np.asarray(x, dtype=np.float32)
    Wj = np.asarray(Wj, dtype=np.float32)
    Wv = np.asarray(Wv, dtype=np.float32)
    Wo = np.asarray(Wo, dtype=np.float32)

    if "nc" not in _cache:
        _cache["nc"] = _build_nc()
    nc = _cache["nc"]

    in_maps = _host_prep(x, Wj, Wv, Wo)
    res = run_bass_kernel_spmd(nc, in_maps, core_ids=list(range(2 * B)))

    out = np.empty((B, T, E), dtype=np.float32)
    for b in range(B):
        acc = res.results[2 * b]["outT"] + res.results[2 * b + 1]["outT"]
        out[b] = acc.T
    return out
